# revision 29
# baseline (speedup 1.0000x reference)
"""Trainium2 Bass kernel for batched multi-head attention (B=8, T=2048, C=1024, H=16).

Sharding: data-parallel over batch — one batch element per NeuronCore (8 cores).

Per-core algorithm (all matmul inputs bf16, accumulation/stats f32):
  qT(j) = (Wq_j^T x^T + bq)   [128, T]  (heads 2j / 2j+1 on partitions 0:64 / 64:128)
  kT(j) = (Wk_j^T xc^T + bk)  [128, TK] (xc = column-compacted x, see below)
  vsb[k, kt, h, 0:64] = v rows scaled by valid(k); col 64 = valid(k)
  S^T[ki, qi] = sum_d kT[d,ki] qT[d,qi]      (PE, K=64, N=512)
  pt = exp(0.125 * S^T)                      (ACT; scores are O(1), no max-sub)
  po[qi, 0:64] += pt_chunk^T @ v_chunk       (PE: lhsT=pt [128k,128q], rhs=vsb
  po[qi, 64]   += row-sum (ones col of vsb)   [128k,65] -> N=65 moving; accumulate
                                              over the key chunks in PSUM)
  out[qi, d] = po[qi, d] * (1/po[qi, 64])    (DVE recip + per-partition scale)
Masked ki slots contribute exactly 0 to both out and l because their vsb row is 0.
The flipped PV produces output directly in [q, d] orientation: no transposes.

K/V positions are compacted to the mask==1 subset on the host and padded to TK
(multiple of 128). With TK == T no compaction happens.

Scheduling: projections are emitted as thunks drained between attention groups
(deadline-driven via ensure_* plus steady prefetch) and PV matmuls are deferred
through a pending FIFO so the PE never sits in-order behind the exp it feeds.
"""

import sys

sys.path.insert(0, "/opt/trn_rl_repo")

from collections import deque
from contextlib import ExitStack

import numpy as np
import ml_dtypes

import concourse.bass as bass  # noqa: F401
import concourse.tile as tile
from concourse import bacc, mybir
from concourse.bass_utils import run_bass_kernel_spmd

B, T, C, H, D = 8, 2048, 1024, 16, 64
NCORES = 8
BF16 = mybir.dt.bfloat16
F32 = mybir.dt.float32

COMPACT = True
TK_COMPACT = 1152

_nc_cache = {}


def build_nc(TK):
    KT = TK // 128
    nc = bacc.Bacc(None)

    xt_d = nc.dram_tensor("xt", [8, 128, T], BF16, kind="ExternalInput")
    sep_xtc = TK != T
    if sep_xtc:
        xtc_d = nc.dram_tensor("xtc", [8, 128, TK], BF16, kind="ExternalInput")
    wqk_d = nc.dram_tensor("wqk", [8, 128, 2048], BF16, kind="ExternalInput")
    wv_d = nc.dram_tensor("wv", [8, 128, 1024], BF16, kind="ExternalInput")
    bqk_d = nc.dram_tensor("bqk", [128, 16], F32, kind="ExternalInput")
    bv_d = nc.dram_tensor("bv", [1, 1024], BF16, kind="ExternalInput")
    mv_d = nc.dram_tensor("mv", [128, KT], F32, kind="ExternalInput")
    out_d = nc.dram_tensor("out", [H * T, D], F32, kind="ExternalOutput")

    with tile.TileContext(nc) as tc, ExitStack() as ctx:
        const = ctx.enter_context(tc.tile_pool(name="const", bufs=1))

        xt = const.tile([128, 8, T], BF16)
        wqk = const.tile([128, 8, 2048], BF16)
        bqk = const.tile([128, 16], F32)
        if sep_xtc:
            xtc = const.tile([128, 8, TK], BF16)
        wv = const.tile([128, 8, 1024], BF16)
        bv = const.tile([1, 1024], BF16)
        mv = const.tile([128, KT], F32)
        ones_r = const.tile([1, 128], BF16)
        vsb = const.tile([128, KT, 16, 65], BF16)

        # ---- input DMAs: one DMA per column-window covering all 8 chunks
        # (strided src AP), ordered so the j=0 pipeline lights up earliest.
        def win_dma(dst, src, w0, w1, c0=0, c1=8):
            nc.sync.dma_start(
                dst[:, c0:c1, w0:w1],
                src[c0:c1, :, w0:w1].rearrange("c p t -> p c t"),
            )

        if sep_xtc:
            win_dma(xtc, xtc_d, 0, 512, 0, 4)
        win_dma(wv, wv_d, 0, 512, 0, 4)
        if sep_xtc:
            win_dma(xtc, xtc_d, 0, 512, 4, 8)
        win_dma(wv, wv_d, 0, 512, 4, 8)
        nc.sync.dma_start(bv[:], bv_d[:])
        nc.sync.dma_start(mv[:], mv_d[:])
        win_dma(xt, xt_d, 0, 512, 0, 4)  # queries window 0
        win_dma(wqk, wqk_d, 0, 512, 0, 4)  # q weights j0-3
        win_dma(xt, xt_d, 0, 512, 4, 8)
        win_dma(wqk, wqk_d, 0, 512, 4, 8)
        win_dma(wqk, wqk_d, 1024, 1536, 0, 4)  # k weights j0-3
        win_dma(wqk, wqk_d, 1024, 1536, 4, 8)
        nc.sync.dma_start(bqk[:], bqk_d[:])
        if sep_xtc:
            for w0 in range(512, TK, 512):
                win_dma(xtc, xtc_d, w0, min(w0 + 512, TK))
        win_dma(wv, wv_d, 512, 1024)
        for w0 in range(512, T, 512):  # remaining query windows
            win_dma(xt, xt_d, w0, w0 + 512)
        win_dma(wqk, wqk_d, 512, 1024)  # q weights j4-7
        win_dma(wqk, wqk_d, 1536, 2048)  # k weights j4-7
        if not sep_xtc:
            xtc = xt

        nc.vector.memset(ones_r[:], 1.0)
        nc.vector.memset(vsb[:, :, :, 64:65], 1.0)

        psum = ctx.enter_context(tc.tile_pool(name="psum", bufs=1, space="PSUM"))
        sb = ctx.enter_context(tc.tile_pool(name="sb", bufs=1))

        # qT/kT ring tiles: chunk j is written during j-1 and read during j.
        qT_t, kT_t = {}, {}

        def get_qT(j):
            if j not in qT_t:
                qT_t[j] = sb.tile([128, T], BF16, tag="qTr", bufs=3, name=f"qT{j}")
            return qT_t[j]

        def get_kT(j):
            if j not in kT_t:
                kT_t[j] = sb.tile([128, TK], BF16, tag="kTr", bufs=3, name=f"kT{j}")
            return kT_t[j]

        # ---- projection thunks (PE matmuls + DVE bias/scale) --------------
        em_q = set()  # (j, tt) emitted
        em_k = {}  # j -> cols emitted
        v_done = [0]  # ti chunks fully emitted

        def q_thunk(j, tt):
            def f():
                qt = get_qT(j)
                ps = psum.tile([128, 1, 512], F32, tag="sp", bufs=1)
                for cc in range(8):
                    nc.tensor.matmul(
                        ps[:, 0, :],
                        wqk[:, cc, j * 128 : (j + 1) * 128],
                        xt[:, cc, tt * 512 : (tt + 1) * 512],
                        start=(cc == 0),
                        stop=(cc == 7),
                    )
                nc.vector.tensor_scalar_add(
                    out=qt[:, tt * 512 : (tt + 1) * 512],
                    in0=ps[:, 0, :],
                    scalar1=bqk[:, j : j + 1],
                )
                em_q.add((j, tt))

            return f

        def k_thunk(j, t0):
            def f():
                kt_ = get_kT(j)
                w = min(512, TK - t0)
                ps = psum.tile([128, 1, 512], F32, tag="sp", bufs=1)
                for cc in range(8):
                    nc.tensor.matmul(
                        ps[:, 0, :w],
                        wqk[:, cc, 1024 + j * 128 : 1024 + (j + 1) * 128],
                        xtc[:, cc, t0 : t0 + w],
                        start=(cc == 0),
                        stop=(cc == 7),
                    )
                nc.vector.tensor_scalar_add(
                    out=kt_[:, t0 : t0 + w],
                    in0=ps[:, 0, :w],
                    scalar1=bqk[:, 8 + j : 9 + j],
                )
                em_k[j] = t0 + w

            return f

        def v_thunk(ti, nn):
            def f():
                ps = psum.tile([128, 1, 512], F32, tag="sp", bufs=1)
                for cc in range(8):
                    nc.tensor.matmul(
                        ps[:, 0, :],
                        xtc[:, cc, ti * 128 : (ti + 1) * 128],
                        wv[:, cc, nn * 512 : (nn + 1) * 512],
                        start=(cc == 0),
                        stop=False,
                    )
                nc.tensor.matmul(
                    ps[:, 0, :],
                    ones_r[:],
                    bv[:, nn * 512 : (nn + 1) * 512],
                    start=False,
                    stop=True,
                )
                nc.vector.tensor_scalar_mul(
                    out=vsb[:, ti, nn * 8 : (nn + 1) * 8, 0:64],
                    in0=ps[:, 0, :].rearrange("p (h d) -> p h d", h=8),
                    scalar1=mv[:, ti : ti + 1],
                )
                if nn == 1:
                    nc.vector.tensor_scalar_mul(
                        out=vsb[:, ti, :, 64:65],
                        in0=vsb[:, ti, :, 64:65],
                        scalar1=mv[:, ti : ti + 1],
                    )
                    v_done[0] = ti + 1

            return f

        items = []
        # j0 extras (q(0,0)/k(0,0) are emitted inline before attention)
        for i, t0 in enumerate(range(512, TK, 512)):
            items.append((i, k_thunk(0, t0)))
        for tt in (1, 2, 3):
            items.append((6 * tt - 2, q_thunk(0, tt)))
        nv = 0
        n_early = min(4, KT)
        for ti in range(n_early):
            items.append((3 + ti, v_thunk(ti, 1)))
        for ti in range(n_early, KT):
            for nn in range(2):
                items.append((7 + nv, v_thunk(ti, nn)))
                nv += 1
        for j in range(1, 8):
            items.append((24 * j - 2, q_thunk(j, 0)))
            for i, t0 in enumerate(range(0, TK, 512)):
                items.append((24 * j - 8 + 2 * i, k_thunk(j, t0)))
            for tt in (1, 2, 3):
                items.append((24 * j + 6 * tt - 4, q_thunk(j, tt)))
        items.sort(key=lambda it: it[0])
        work = deque(items)

        def drain(n):
            while n > 0 and work:
                work.popleft()[1]()
                n -= 1

        def drain_due(g):
            while work and work[0][0] <= g:
                work.popleft()[1]()

        def ensure_v(ti):
            while v_done[0] < ti + 1:
                assert work, "work exhausted before v ready"
                drain(1)

        def ensure_proj(j, qi, kcols):
            while (j, qi) not in em_q or em_k.get(j, 0) < kcols:
                assert work, "work exhausted before proj ready"
                drain(1)

        # ---- deferred-work FIFO ------------------------------------------
        pending = deque()
        depth = [1]

        def pump():
            while len(pending) > depth[0]:
                pending.popleft()()

        def flush():
            while pending:
                pending.popleft()()

        def pv_closure(po, pt, kts, h):
            def f():
                ensure_v(kts[-1])
                # One PSUM accumulation group spans the whole bank: start only
                # on the first matmul (zeroes the 2KB region), stop on the
                # last. Each qc chain's first write overwrites pending-zero
                # bytes, later writes accumulate.
                for idx, kt in enumerate(kts):
                    for qc in range(4):
                        nc.tensor.matmul(
                            po[:, qc, 0:65],
                            pt[:, idx, qc * 128 : (qc + 1) * 128],
                            vsb[:, kt, h, :],
                            start=(kt == 0 and qc == 0),
                            stop=(kt == KT - 1 and qc == 3),
                        )

            return f

        def finish_closure(po, stage):
            def f():
                rc = sb.tile([128, 4, 1], F32, tag="rc", bufs=3, name="rc")
                nc.vector.reciprocal(rc[:], po[:, :, 64:65])
                nc.vector.tensor_mul(
                    stage[:, :, :],
                    po[:, :, 0:64],
                    rc[:, :, 0:1].broadcast_to([128, 4, 64]),
                )

            return f

        def dma_closure(stage, j, h_loc, qi):
            def f():
                h = 2 * j + h_loc
                base = h * T + qi * 512
                nc.sync.dma_start(
                    out_d[base : base + 512, :].rearrange("(qc p) d -> p qc d", p=128),
                    stage[:],
                )

            return f

        # ---- attention ----------------------------------------------------
        # Front-load v-projection into the DMA-bound head window.
        v_thunk(0, 0)()
        v_thunk(1, 0)()
        v_thunk(2, 0)()
        q_thunk(0, 0)()
        v_thunk(3, 0)()
        k_thunk(0, 0)()

        GRP = 3
        n_grp = (KT + GRP - 1) // GRP
        gctr = 0  # global group counter for prefetch pacing

        worder = [(jj, qi, hl) for jj in range(8) for qi in range(4) for hl in (0, 1)]

        for widx, (j, qi, h_loc) in enumerate(worder):
            if True:
                if True:
                    h = 2 * j + h_loc
                    prt = slice(64 * h_loc, 64 * (h_loc + 1))
                    stage = sb.tile(
                        [128, 4, 64], F32, tag=f"stage{h_loc}", bufs=4,
                        name=f"stg{h_loc}",
                    )
                    po = psum.tile([128, 4, 128], F32, tag="po", bufs=1)
                    for g in range(n_grp):
                        kts = list(range(g * GRP, min((g + 1) * GRP, KT)))
                        gl = len(kts)
                        ensure_proj(j, qi, kts[-1] * 128 + 128)
                        ps = psum.tile([128, 3, 512], F32, tag="s", bufs=2)
                        qt, kt_ = get_qT(j), get_kT(j)
                        for idx, kt in enumerate(kts):
                            nc.tensor.matmul(
                                ps[:, idx, :],
                                kt_[prt, kt * 128 : (kt + 1) * 128],
                                qt[prt, qi * 512 : (qi + 1) * 512],
                                start=True,
                                stop=True,
                            )
                        gctr += 1
                        depth[0] = 18 if gctr <= 18 else 6
                        if work and work[0][0] <= gctr:
                            work.popleft()[1]()
                        pt = sb.tile([128, 3, 512], BF16, tag="pt", bufs=20)
                        nc.scalar.activation(
                            out=pt[:, :gl, :],
                            in_=ps[:, :gl, :],
                            func=mybir.ActivationFunctionType.Exp,
                            scale=0.125,
                        )
                        pending.append(pv_closure(po, pt, kts, h))
                        pump()
                        drain_due(gctr)
                    pending.append(finish_closure(po, stage))
                    pending.append(dma_closure(stage, j, h_loc, qi))
        flush()

    nc.compile()
    return nc


def _prep_core(xb, maskb, W_bf, Wv_bf, bqk_np, bv_np, TK):
    """Build the per-core input map for batch element xb (T, C), maskb (T,)."""
    xTb = np.ascontiguousarray(xb.T)  # (C, T) f32
    xt = xTb.astype(ml_dtypes.bfloat16).reshape(8, 128, T)
    m = {"xt": xt, "wqk": W_bf, "wv": Wv_bf, "bqk": bqk_np, "bv": bv_np}
    if TK == T:
        mvv = maskb.astype(np.float32).reshape(TK // 128, 128).T.copy()  # (128, KT)
    else:
        sel = np.nonzero(maskb)[0]
        assert len(sel) <= TK, f"compaction overflow: {len(sel)} > {TK}"
        xc = np.zeros((C, TK), np.float32)
        xc[:, : len(sel)] = xTb[:, sel]
        m["xtc"] = xc.astype(ml_dtypes.bfloat16).reshape(8, 128, TK)
        mvv = np.zeros(TK, np.float32)
        mvv[: len(sel)] = 1.0
        mvv = mvv.reshape(TK // 128, 128).T.copy()
    m["mv"] = mvv
    return m


def make_in_maps(x, mask, W_qkv, b_qkv, TK):
    W_bf = np.ascontiguousarray(W_qkv[:, :2048]).astype(ml_dtypes.bfloat16).reshape(8, 128, 2048)
    Wv_bf = np.ascontiguousarray(W_qkv[:, 2048:]).astype(ml_dtypes.bfloat16).reshape(8, 128, 1024)
    bqk_np = b_qkv[:2048].astype(np.float32).reshape(16, 128).T.copy()  # (128, 16)
    bv_np = b_qkv[2048:].astype(np.float32).reshape(1, 1024).astype(ml_dtypes.bfloat16)
    mask2 = np.asarray(mask).reshape(B, T)
    return [
        _prep_core(np.asarray(x[b]), mask2[b], W_bf, Wv_bf, bqk_np, bv_np, TK)
        for b in range(B)
    ]


def kernel(x, mask, W_qkv, b_qkv):
    mask2 = np.asarray(mask).reshape(B, T)
    TK = T
    if COMPACT:
        need = int(max(mask2.sum(axis=1)))
        if need <= TK_COMPACT:
            TK = TK_COMPACT
    if TK not in _nc_cache:
        _nc_cache[TK] = build_nc(TK)
    nc = _nc_cache[TK]
    in_maps = make_in_maps(x, mask, W_qkv, b_qkv, TK)
    res = run_bass_kernel_spmd(nc, in_maps, core_ids=list(range(NCORES)))
    out = np.stack([res.results[c]["out"] for c in range(NCORES)])  # (B, H*T, D)
    return out.reshape(B, T, C)


if __name__ == "__main__":
    rng = np.random.default_rng(0)
    x = rng.standard_normal((B, T, C), dtype=np.float32)
    mask = (rng.integers(0, 2, (B, 1, 1, T))).astype(np.int32)
    W = (rng.standard_normal((C, 3 * C), dtype=np.float32) * C**-0.5).astype(np.float32)
    bq = (rng.standard_normal(3 * C, dtype=np.float32) * 0.02).astype(np.float32)
    out = kernel(x, mask, W, bq)
    print("out", out.shape, out.dtype)
